# revision 1
# baseline (speedup 1.0000x reference)
"""CRF loss (nn_ConditionalRandomField) Bass/Trainium2 kernel.

Strategy
--------
loss = sum_b (numerator[b] - log_partition[b])

- log_partition (the B*T*N^2 forward scan — 99.99% of FLOPs) runs on 8
  NeuronCores, data-parallel over the batch dim (32 sequences/core).
- The scan runs in exp-space: A_t = (exp(trans)^T_pe @ A_{t-1}) * exp(emit_t),
  one PE matmul chain per step (bf16 inputs, f32 PSUM accumulation), with a
  sum-renormalization every 8 steps to stay in fp32/bf16 dynamic range.
  Renorm bookkeeping is exact: the actually-applied factor r (f32) has its
  log stashed on-chip and added back at the end.
- Layout: tag dim N=256 on partitions (2 tiles of 128), batch on the free
  dim. Host pre-transposes inputs to (N, T, B_core) per core — pure layout
  glue so DMA loads are contiguous. exp() of inputs runs on-device (ACT).
- numerator is a tiny O(B*T) gather -- computed on host in numpy.
"""

import numpy as np

B, T, N = 256, 512, 256
START, STOP = 254, 255
NCORES = 8
BC = B // NCORES  # 32 sequences per core
K_RENORM = 8


def _build_program(t_steps=T, chunk=64):
    """Build + compile the single-core SPMD Bass program."""
    import concourse.bass as bass
    import concourse.tile as tile
    from concourse import bacc, mybir

    f32 = mybir.dt.float32
    bf16 = mybir.dt.bfloat16
    EXP = mybir.ActivationFunctionType.Exp
    LN = mybir.ActivationFunctionType.Ln

    n_chunks = (t_steps + chunk - 1) // chunk
    assert t_steps % chunk == 0 or n_chunks * chunk >= t_steps
    renorm_ts = [t for t in range(1, t_steps - 1) if t % K_RENORM == K_RENORM - 1]
    n_renorm = len(renorm_ts)
    assert n_renorm <= 64

    nc = bacc.Bacc("TRN2", target_bir_lowering=False, debug=False,
                   enable_asserts=False)

    # DRAM I/O. x is the per-core input shard pre-transposed on host to
    # (n_tile, 128, T, BC) f32; transT is transitions.T (contiguous);
    # startcol/stopcol are trans[:, START] / trans[STOP, :] as columns.
    x = nc.dram_tensor("x", [2, 128, t_steps, BC], f32, kind="ExternalInput").ap()
    transT = nc.dram_tensor("transT", [2, 128, 256], f32, kind="ExternalInput").ap()
    startcol = nc.dram_tensor("startcol", [2, 128, 1], f32, kind="ExternalInput").ap()
    stopcol = nc.dram_tensor("stopcol", [2, 128, 1], f32, kind="ExternalInput").ap()
    denom_out = nc.dram_tensor("denom", [1, BC], f32, kind="ExternalOutput").ap()

    with tile.TileContext(nc) as tc:
        with (
            tc.tile_pool(name="consts", bufs=1) as consts,
            tc.tile_pool(name="wstage", bufs=1) as wstage,
            tc.tile_pool(name="ebig", bufs=1) as ebig,
            tc.tile_pool(name="stg", bufs=3) as stg,
            tc.tile_pool(name="apool", bufs=3) as apool,
            tc.tile_pool(name="tmp", bufs=2) as tmpp,
            tc.tile_pool(name="fin", bufs=1) as fin,
            tc.tile_pool(name="ps", bufs=3, space="PSUM") as psp,
            tc.tile_pool(name="pssm", bufs=1, space="PSUM") as pssm,
            tc.tile_pool(name="psb", bufs=1, space="PSUM") as psbp,
        ):
            # ---- constants ----
            ones128_bf = consts.tile([128, 1], bf16)
            nc.vector.memset(ones128_bf, 1.0)
            ones_row = consts.tile([1, 128], bf16)
            nc.vector.memset(ones_row, 1.0)
            # stash holds the raw renorm factors r_j (~2^-75); unwritten
            # slots stay 2^-64 so the finale Ln(r * 2^64) gives exactly 0.
            stash = consts.tile([1, BC, 64], f32)
            nc.vector.memset(stash, float(2.0 ** -64))

            # ---- weights: exp(transT) as bf16, 2 k-tiles of [128, 256] ----
            # clamp the -10000 sentinels to -100 before Exp: the ACT spline
            # tables only cover a limited input range; exp(-100) is still
            # exactly 0 in bf16/f32 after rounding of products.
            wtiles = []
            for k in range(2):
                wstg = wstage.tile([128, 256], f32, tag=f"wstg{k}")
                nc.sync.dma_start(out=wstg, in_=transT[k])
                nc.vector.tensor_scalar_max(wstg, wstg, -100.0)
                wt = consts.tile([128, 256], bf16, tag=f"w{k}")
                nc.scalar.activation(out=wt, in_=wstg, func=EXP)
                wtiles.append(wt)

            expstart, expstop = [], []
            for k in range(2):
                sstg = wstage.tile([128, 1], f32, tag=f"sstg{k}")
                nc.sync.dma_start(out=sstg, in_=startcol[k])
                nc.vector.tensor_scalar_max(sstg, sstg, -100.0)
                es = consts.tile([128, 1], f32, tag=f"es{k}")
                nc.scalar.activation(out=es, in_=sstg, func=EXP)
                expstart.append(es)
                pstg = wstage.tile([128, 1], f32, tag=f"pstg{k}")
                nc.sync.dma_start(out=pstg, in_=stopcol[k])
                nc.vector.tensor_scalar_max(pstg, pstg, -100.0)
                ep = consts.tile([128, 1], f32, tag=f"ep{k}")
                nc.scalar.activation(out=ep, in_=pstg, func=EXP)
                expstop.append(ep)

            # ---- stream inputs: DMA f32 chunk -> ACT exp -> bf16 E tiles ----
            echunks = [[None] * n_chunks for _ in range(2)]
            for c in range(n_chunks):
                t0 = c * chunk
                t1 = min(t0 + chunk, t_steps)
                for j in range(2):
                    s = stg.tile([128, t1 - t0, BC], f32, tag=f"stg{j}")
                    nc.sync.dma_start(out=s, in_=x[j, :, t0:t1, :])
                    e = ebig.tile([128, t1 - t0, BC], bf16, tag=f"e{j}c{c}")
                    nc.scalar.activation(out=e, in_=s, func=EXP)
                    echunks[j][c] = e

            def eslice(j, t):
                return echunks[j][t // chunk][:, t % chunk, :]

            # ---- scan ----
            a_prev = []
            for j in range(2):
                a0 = apool.tile([128, BC], bf16, tag=f"a{j}")
                nc.vector.tensor_scalar_mul(a0, eslice(j, 0), expstart[j])
                a_prev.append(a0)

            pending_bcast = None
            n_stashed = 0
            for t in range(1, t_steps):
                a_new = []
                for j in range(2):
                    ps = psp.tile([128, BC], f32, tag=f"ps{j}")
                    nc.tensor.matmul(ps, wtiles[0][:, j * 128:(j + 1) * 128],
                                     a_prev[0], start=True, stop=False)
                    nc.tensor.matmul(ps, wtiles[1][:, j * 128:(j + 1) * 128],
                                     a_prev[1], start=False, stop=True)
                    an = apool.tile([128, BC], bf16, tag=f"a{j}")
                    if pending_bcast is None:
                        nc.vector.tensor_mul(an, ps, eslice(j, t))
                    else:
                        tm = tmpp.tile([128, BC], f32, tag=f"tm{j}")
                        nc.vector.tensor_mul(tm, ps, eslice(j, t))
                        nc.vector.tensor_mul(an, tm, pending_bcast)
                    a_new.append(an)
                pending_bcast = None
                a_prev = a_new

                if t in renorm_ts:
                    pss = pssm.tile([1, BC], f32, tag="pss")
                    nc.tensor.matmul(pss, ones128_bf, a_prev[0],
                                     start=True, stop=False)
                    nc.tensor.matmul(pss, ones128_bf, a_prev[1],
                                     start=False, stop=True)
                    r = tmpp.tile([1, BC], f32, tag="recip")
                    nc.vector.reciprocal_approx_fast(r, pss)
                    rbf = tmpp.tile([1, BC], bf16, tag="recipbf")
                    nc.vector.tensor_copy(rbf, r)
                    pb = psbp.tile([128, BC], f32, tag="pb")
                    nc.tensor.matmul(pb, ones_row, rbf, start=True, stop=True)
                    # stash the actually-applied factor raw (exact
                    # bookkeeping); one Ln pass over all of them at the
                    # finale avoids Exp<->Ln ACT table thrash in the scan.
                    nc.vector.tensor_copy(stash[0:1, :, n_stashed], pb[0:1, :])
                    n_stashed += 1
                    pending_bcast = pb

            # ---- finale ----
            # one last renorm so the stop-weighted sum stays within ACT Ln's
            # valid input range (A_L alone reaches ~2^79)
            pss = pssm.tile([1, BC], f32, tag="pss")
            nc.tensor.matmul(pss, ones128_bf, a_prev[0], start=True, stop=False)
            nc.tensor.matmul(pss, ones128_bf, a_prev[1], start=False, stop=True)
            r = tmpp.tile([1, BC], f32, tag="recip")
            nc.vector.reciprocal_approx_fast(r, pss)
            rbf = tmpp.tile([1, BC], bf16, tag="recipbf")
            nc.vector.tensor_copy(rbf, r)
            pb = psbp.tile([128, BC], f32, tag="pb")
            nc.tensor.matmul(pb, ones_row, rbf, start=True, stop=True)
            nc.vector.tensor_copy(stash[0:1, :, n_stashed], pb[0:1, :])
            n_stashed += 1

            astop = []
            for j in range(2):
                af1 = tmpp.tile([128, BC], f32, tag=f"tm{j}")
                nc.vector.tensor_scalar_mul(af1, a_prev[j], expstop[j])
                af = fin.tile([128, BC], bf16, tag=f"astop{j}")
                nc.vector.tensor_mul(af, af1, pb)
                astop.append(af)
            psw = pssm.tile([1, BC], f32, tag="pss")
            nc.tensor.matmul(psw, ones128_bf, astop[0], start=True, stop=False)
            nc.tensor.matmul(psw, ones128_bf, astop[1], start=False, stop=True)
            # Ln(r * 2^64) for all stashed factors in one ACT op (r ~ 2^-75
            # is below the Ln spline's ~[2^-64, 2^64] domain; the exact p2
            # scale recenters it, undone via `corr` below).
            stashln = fin.tile([1, BC, 64], f32, tag="stashln")
            nc.scalar.activation(out=stashln, in_=stash, func=LN,
                                 scale=float(2.0 ** 64))
            logsum = fin.tile([1, BC], f32, tag="logsum")
            import concourse.mybir as _mybir
            nc.vector.reduce_sum(logsum, stashln, axis=_mybir.AxisListType.X)
            lnw = fin.tile([1, BC], f32, tag="lnw")
            nc.scalar.activation(out=lnw, in_=psw, func=LN)
            dn = fin.tile([1, BC], f32, tag="dn")
            nc.vector.tensor_sub(dn, lnw, logsum)
            # undo the 2^64 scale applied inside each stashed Ln
            corr = float(n_stashed * 64.0 * np.log(2.0))
            dn2 = fin.tile([1, BC], f32, tag="dn2")
            nc.vector.tensor_scalar_add(dn2, dn, corr)
            nc.sync.dma_start(out=denom_out, in_=dn2)

    nc.compile()
    return nc


_PROG_CACHE = {}


def _get_program(t_steps=T, chunk=64):
    key = (t_steps, chunk)
    if key not in _PROG_CACHE:
        _PROG_CACHE[key] = _build_program(t_steps, chunk)
    return _PROG_CACHE[key]


def _host_numerator(inputs, transitions, tags, mask):
    fm = mask.astype(np.float32)
    score = transitions[tags[:, 0], START].astype(np.float32)
    trans_sc = transitions[tags[:, 1:], tags[:, :-1]] * fm[:, 1:]
    emit_sc = np.take_along_axis(
        inputs[:, :-1, :], tags[:, :-1, None], axis=2)[..., 0] * fm[:, :-1]
    score = score + trans_sc.sum(-1) + emit_sc.sum(-1)
    last_idx = (fm.sum(-1) - 1.0).astype(np.int32)
    last_tags = np.take_along_axis(tags, last_idx[:, None], axis=1)[:, 0]
    last_input = np.take_along_axis(
        inputs[:, -1, :], last_tags[:, None], axis=1)[:, 0]
    return score + transitions[STOP, last_tags] + last_input * fm[:, -1]


def _make_in_maps(inputs, transitions):
    xt = np.ascontiguousarray(inputs.transpose(2, 1, 0))  # (N, T, B) f32
    transT = np.ascontiguousarray(transitions.T).reshape(2, 128, 256)
    sc = np.ascontiguousarray(transitions[:, START]).reshape(2, 128, 1)
    st = np.ascontiguousarray(transitions[STOP, :]).reshape(2, 128, 1)
    in_maps = []
    for c in range(NCORES):
        xc = np.ascontiguousarray(
            xt[:, :, c * BC:(c + 1) * BC]).reshape(2, 128, xt.shape[1], BC)
        in_maps.append({"x": xc, "transT": transT,
                        "startcol": sc, "stopcol": st})
    return in_maps


def kernel(inputs, transitions, tags, mask, _trace=False):
    from concourse.bass_utils import run_bass_kernel_spmd

    inputs = np.asarray(inputs, dtype=np.float32)
    transitions = np.asarray(transitions, dtype=np.float32)
    tags = np.asarray(tags)
    mask = np.asarray(mask)

    nc = _get_program()
    in_maps = _make_in_maps(inputs, transitions)
    res = run_bass_kernel_spmd(nc, in_maps, list(range(NCORES)), trace=_trace)
    denoms = np.concatenate([r["denom"].reshape(-1) for r in res.results])

    num = _host_numerator(inputs, transitions, tags, mask)
    out = np.float32(np.sum(num.astype(np.float64) - denoms.astype(np.float64)))
    if _trace:
        return out, res
    return out



# revision 2
# speedup vs baseline: 2.1396x; 2.1396x over previous
"""CRF loss (nn_ConditionalRandomField) Bass/Trainium2 kernel — v2.

Strategy
--------
loss = sum_b (numerator[b] - log_denominator[b])

- The denominator scan (B*T*N^2 — 99.99% of FLOPs) runs on 8 NeuronCores,
  data-parallel over batch (32 sequences/core), in exp space:
      A_t = (W @ A_{t-1}) * E_t          (forward)
  with W = exp(transitions), E_t = exp(inputs_t) * 2^-C (constant
  per-step prescale C≈9.42 bits — matches the mean log-growth, so the
  state drifts only +-14 bits over 512 steps; no data-dependent
  renormalization needed; the exact correction 512*C*ln2 is added back
  on the host).
- Time is split fwd/bwd: cores run the forward recursion 0..255 AND the
  backward (beta) recursion 511..256 as two independent chains that meet
  at t=255 (denominator = ln(beta_255 . A_255)). 256 sequential rounds
  instead of 511, and the two chains keep PE/DVE pipelined.
- Layout: tag dim on partitions split in two halves j=0/1; the state is
  ONE [128, 64] tile per chain with columns (j*32 + b), so each round is
  4 matmuls (2 output-halves x 2 k-tiles, FD=32) into a single [128,64]
  PSUM tile + ONE vector multiply (PSUM-access fixed cost is paid once).
- Weights are fp8e4 (exp'd on host): LDWEIGHTS runs 4x faster via fast
  weight load, which otherwise dominates PE time at FD=32.
- E tiles are exp'd + prescaled on host, shipped bf16, DMA'd in chunks.
- numerator is a tiny O(B*T) gather — computed on host in numpy.
"""

import numpy as np
import ml_dtypes

B, T, N = 256, 512, 256
START, STOP = 254, 255
NCORES = 8
BC = B // NCORES          # 32 sequences per core
R = 256                   # rounds (fwd steps 1..255, bwd steps 511..256)
CH = 32                   # E-chunk size in rounds
SCALE_BITS = 9.42         # per-step 2^-C prescale folded into E on host
LN2 = float(np.log(2.0))


def _build_program():
    import concourse.bass as bass  # noqa: F401
    import concourse.tile as tile
    from concourse import bacc, mybir

    f32 = mybir.dt.float32
    bf16 = mybir.dt.bfloat16
    fp8 = mybir.dt.float8e4

    nch = R // CH

    nc = bacc.Bacc("TRN2", target_bir_lowering=False, debug=False,
                   enable_asserts=False)

    # DRAM I/O (per core). ef/eb are host-exp'd prescaled emissions:
    #   ef[p, r, j*32+b] = exp(x[b, r,     j*128+p]) * 2^-C   (r = 0..255)
    #   eb[p, r, j*32+b] = exp(x[b, 511-r, j*128+p]) * 2^-C
    # wf/wb are the 8 stationary fp8 weight tiles (see _make_in_maps).
    # esx/stx are exp(trans[:, START]) / exp(trans[STOP, :]) halves.
    ef = nc.dram_tensor("ef", [128, R, 2 * BC], bf16, kind="ExternalInput").ap()
    eb = nc.dram_tensor("eb", [128, R, 2 * BC], bf16, kind="ExternalInput").ap()
    wf = nc.dram_tensor("wf", [4, 128, 128], fp8, kind="ExternalInput").ap()
    wb = nc.dram_tensor("wb", [4, 128, 128], fp8, kind="ExternalInput").ap()
    esx = nc.dram_tensor("esx", [2, 128, 1], f32, kind="ExternalInput").ap()
    stx = nc.dram_tensor("stx", [2, 128, 1], f32, kind="ExternalInput").ap()
    sums_out = nc.dram_tensor("sums", [1, 2 * BC], f32, kind="ExternalOutput").ap()

    with tile.TileContext(nc) as tc:
        with (
            tc.tile_pool(name="consts", bufs=1) as consts,
            tc.tile_pool(name="ebig", bufs=1) as ebig,
            tc.tile_pool(name="afp", bufs=3) as afp,
            tc.tile_pool(name="bxp", bufs=3) as bxp,
            tc.tile_pool(name="fin", bufs=1) as fin,
            tc.tile_pool(name="psf", bufs=3, space="PSUM") as psfp,
            tc.tile_pool(name="psb", bufs=3, space="PSUM") as psbp,
            tc.tile_pool(name="pssm", bufs=1, space="PSUM") as pssm,
        ):
            # ---- constants / weights ----
            ones128_bf = consts.tile([128, 1], bf16)
            nc.vector.memset(ones128_bf, 1.0)

            wtf, wtb = [], []
            for i in range(4):
                w = consts.tile([128, 128], fp8, tag=f"wf{i}")
                nc.sync.dma_start(out=w, in_=wf[i])
                wtf.append(w)
                w = consts.tile([128, 128], fp8, tag=f"wb{i}")
                nc.sync.dma_start(out=w, in_=wb[i])
                wtb.append(w)

            es, st = [], []
            for j in range(2):
                t_ = consts.tile([128, 1], f32, tag=f"es{j}")
                nc.sync.dma_start(out=t_, in_=esx[j])
                es.append(t_)
                t_ = consts.tile([128, 1], f32, tag=f"st{j}")
                nc.sync.dma_start(out=t_, in_=stx[j])
                st.append(t_)

            # ---- E chunks: straight DMA into persistent bf16 tiles ----
            efch, ebch = [None] * nch, [None] * nch
            for c in range(nch):
                t0 = c * CH
                e_ = ebig.tile([128, CH, 2 * BC], bf16, tag=f"ef{c}")
                nc.sync.dma_start(out=e_, in_=ef[:, t0:t0 + CH, :])
                efch[c] = e_
                e_ = ebig.tile([128, CH, 2 * BC], bf16, tag=f"eb{c}")
                nc.sync.dma_start(out=e_, in_=eb[:, t0:t0 + CH, :])
                ebch[c] = e_

            def efs(r):
                return efch[r // CH][:, r % CH, :]

            def ebs(r):
                return ebch[r // CH][:, r % CH, :]

            # ---- init states ----
            # A_0 = E'_0 * exp(trans[:, START])  (per tag-half j)
            af = afp.tile([128, 2 * BC], bf16, tag="af")
            for j in range(2):
                nc.vector.tensor_scalar_mul(
                    af[:, j * BC:(j + 1) * BC], efs(0)[:, j * BC:(j + 1) * BC], es[j])
            # X_0 = beta_511 * E'_511 = stop * E'b[0]
            bx = bxp.tile([128, 2 * BC], bf16, tag="bx")
            for j in range(2):
                nc.vector.tensor_scalar_mul(
                    bx[:, j * BC:(j + 1) * BC], ebs(0)[:, j * BC:(j + 1) * BC], st[j])

            # ---- scan: 256 rounds, two chains ----
            for r in range(1, R + 1):
                # bwd: beta = W^T @ X   (consumes bx = X^{(r-1)})
                psb = psbp.tile([128, 2 * BC], f32, tag="psb")
                for jo in range(2):
                    o = psb[:, jo * BC:(jo + 1) * BC]
                    nc.tensor.matmul(o, wtb[0 * 2 + jo], bx[:, 0:BC],
                                     start=True, stop=False)
                    nc.tensor.matmul(o, wtb[1 * 2 + jo], bx[:, BC:2 * BC],
                                     start=False, stop=True)
                if r <= R - 1:
                    # fwd: P = W @ A_{r-1}
                    psf = psfp.tile([128, 2 * BC], f32, tag="psf")
                    for jo in range(2):
                        o = psf[:, jo * BC:(jo + 1) * BC]
                        nc.tensor.matmul(o, wtf[0 * 2 + jo], af[:, 0:BC],
                                         start=True, stop=False)
                        nc.tensor.matmul(o, wtf[1 * 2 + jo], af[:, BC:2 * BC],
                                         start=False, stop=True)
                    af_new = afp.tile([128, 2 * BC], bf16, tag="af")
                    nc.vector.tensor_mul(af_new, psf, efs(r))
                    af = af_new
                    bx_new = bxp.tile([128, 2 * BC], bf16, tag="bx")
                    nc.vector.tensor_mul(bx_new, psb, ebs(r))
                    bx = bx_new
                else:
                    # join: S_jb = sum_p A_255 * beta_255   (beta_255 = psb)
                    tj = fin.tile([128, 2 * BC], bf16, tag="tj")
                    nc.vector.tensor_mul(tj, psb, af)
                    sums_ps = pssm.tile([1, 2 * BC], f32, tag="sums")
                    nc.tensor.matmul(sums_ps, ones128_bf, tj,
                                     start=True, stop=True)
                    sums_sb = fin.tile([1, 2 * BC], f32, tag="sums_sb")
                    nc.vector.tensor_copy(sums_sb, sums_ps)
                    nc.sync.dma_start(out=sums_out, in_=sums_sb)

    nc.compile()
    return nc


_PROG_CACHE = {}


def _get_program():
    if "p" not in _PROG_CACHE:
        _PROG_CACHE["p"] = _build_program()
    return _PROG_CACHE["p"]


def _host_numerator(inputs, transitions, tags, mask):
    fm = mask.astype(np.float32)
    score = transitions[tags[:, 0], START].astype(np.float32)
    trans_sc = transitions[tags[:, 1:], tags[:, :-1]] * fm[:, 1:]
    emit_sc = np.take_along_axis(
        inputs[:, :-1, :], tags[:, :-1, None], axis=2)[..., 0] * fm[:, :-1]
    score = score + trans_sc.sum(-1) + emit_sc.sum(-1)
    last_idx = (fm.sum(-1) - 1.0).astype(np.int32)
    last_tags = np.take_along_axis(tags, last_idx[:, None], axis=1)[:, 0]
    last_input = np.take_along_axis(
        inputs[:, -1, :], last_tags[:, None], axis=1)[:, 0]
    return score + transitions[STOP, last_tags] + last_input * fm[:, -1]


def _make_in_maps(inputs, transitions):
    bf = ml_dtypes.bfloat16
    fp8 = ml_dtypes.float8_e4m3

    # E' = exp(x - C*ln2) as bf16, laid out [tag_part, round, (j, b)]
    ex = np.exp(inputs.astype(np.float32) - np.float32(SCALE_BITS * LN2))
    # v[j, p, t, b] with tag = j*128 + p
    v = ex.transpose(2, 1, 0).reshape(2, 128, T, B)
    tc = np.maximum(transitions, -100.0).astype(np.float32)
    expt = np.exp(tc)  # W[next, prev]
    # fwd lhsT (k*2+j): W^T slice  [prev k-half, next j-half]
    wfs = np.ascontiguousarray(
        expt.T.reshape(2, 128, 2, 128).transpose(0, 2, 1, 3)
    ).reshape(4, 128, 128).astype(fp8)
    # bwd lhsT (k*2+j): W slice    [next k-half, prev j-half]
    wbs = np.ascontiguousarray(
        expt.reshape(2, 128, 2, 128).transpose(0, 2, 1, 3)
    ).reshape(4, 128, 128).astype(fp8)
    esx = np.exp(np.maximum(transitions[:, START], -100.0)
                 ).astype(np.float32).reshape(2, 128, 1)
    stx = np.exp(np.maximum(transitions[STOP, :], -100.0)
                 ).astype(np.float32).reshape(2, 128, 1)

    in_maps = []
    for c in range(NCORES):
        vc = v[:, :, :, c * BC:(c + 1) * BC]          # [2, 128, T, BC]
        efc = np.ascontiguousarray(
            vc[:, :, 0:R, :].transpose(1, 2, 0, 3)).reshape(128, R, 2 * BC)
        ebc = np.ascontiguousarray(
            vc[:, :, T - 1:R - 1:-1, :].transpose(1, 2, 0, 3)
        ).reshape(128, R, 2 * BC)
        in_maps.append({
            "ef": efc.astype(bf), "eb": ebc.astype(bf),
            "wf": wfs, "wb": wbs, "esx": esx, "stx": stx,
        })
    return in_maps


def kernel(inputs, transitions, tags, mask, _trace=False):
    from concourse.bass_utils import run_bass_kernel_spmd

    inputs = np.asarray(inputs, dtype=np.float32)
    transitions = np.asarray(transitions, dtype=np.float32)
    tags = np.asarray(tags)
    mask = np.asarray(mask)

    nc = _get_program()
    in_maps = _make_in_maps(inputs, transitions)
    res = run_bass_kernel_spmd(nc, in_maps, list(range(NCORES)), trace=_trace)
    sums = np.stack([r["sums"].reshape(2, BC) for r in res.results])  # (8,2,BC)
    S = (sums[:, 0, :] + sums[:, 1, :]).reshape(-1).astype(np.float64)
    denoms = np.log(S) + (2 * R) * SCALE_BITS * LN2

    num = _host_numerator(inputs, transitions, tags, mask).astype(np.float64)
    out = np.float32(np.sum(num - denoms))
    if _trace:
        return out, res
    return out


# revision 3
# speedup vs baseline: 2.1471x; 1.0035x over previous
"""CRF loss (nn_ConditionalRandomField) Bass/Trainium2 kernel — v2.

Strategy
--------
loss = sum_b (numerator[b] - log_denominator[b])

- The denominator scan (B*T*N^2 — 99.99% of FLOPs) runs on 8 NeuronCores,
  data-parallel over batch (32 sequences/core), in exp space:
      A_t = (W @ A_{t-1}) * E_t          (forward)
  with W = exp(transitions), E_t = exp(inputs_t) * 2^-C (constant
  per-step prescale C≈9.42 bits — matches the mean log-growth, so the
  state drifts only +-14 bits over 512 steps; no data-dependent
  renormalization needed; the exact correction 512*C*ln2 is added back
  on the host).
- Time is split fwd/bwd: cores run the forward recursion 0..255 AND the
  backward (beta) recursion 511..256 as two independent chains that meet
  at t=255 (denominator = ln(beta_255 . A_255)). 256 sequential rounds
  instead of 511, and the two chains keep PE/DVE pipelined.
- Layout: tag dim on partitions split in two halves j=0/1; the state is
  ONE [128, 64] tile per chain with columns (j*32 + b), so each round is
  4 matmuls (2 output-halves x 2 k-tiles, FD=32) into a single [128,64]
  PSUM tile + ONE vector multiply (PSUM-access fixed cost is paid once).
- Weights are fp8e4 (exp'd on host): LDWEIGHTS runs 4x faster via fast
  weight load, which otherwise dominates PE time at FD=32.
- E tiles are exp'd + prescaled on host, shipped bf16, DMA'd in chunks.
- numerator is a tiny O(B*T) gather — computed on host in numpy.
"""

import numpy as np
import ml_dtypes

B, T, N = 256, 512, 256
START, STOP = 254, 255
NCORES = 8
BC = B // NCORES          # 32 sequences per core
R = 256                   # rounds (fwd steps 1..255, bwd steps 511..256)
CH = 32                   # E-chunk size in rounds
SCALE_BITS = 9.42         # per-step 2^-C prescale folded into E on host
LN2 = float(np.log(2.0))


def _build_program():
    import concourse.bass as bass  # noqa: F401
    import concourse.tile as tile
    from concourse import bacc, mybir

    f32 = mybir.dt.float32
    bf16 = mybir.dt.bfloat16
    fp8 = mybir.dt.float8e4

    nch = R // CH

    nc = bacc.Bacc("TRN2", target_bir_lowering=False, debug=False,
                   enable_asserts=False)

    # DRAM I/O (per core). ef/eb are host-exp'd prescaled emissions:
    #   ef[p, r, j*32+b] = exp(x[b, r,     j*128+p]) * 2^-C   (r = 0..255)
    #   eb[p, r, j*32+b] = exp(x[b, 511-r, j*128+p]) * 2^-C
    # wf/wb are the 8 stationary fp8 weight tiles (see _make_in_maps).
    # esx/stx are exp(trans[:, START]) / exp(trans[STOP, :]) halves.
    ef = nc.dram_tensor("ef", [128, R, 2 * BC], bf16, kind="ExternalInput").ap()
    eb = nc.dram_tensor("eb", [128, R, 2 * BC], bf16, kind="ExternalInput").ap()
    wf = nc.dram_tensor("wf", [4, 128, 128], fp8, kind="ExternalInput").ap()
    wb = nc.dram_tensor("wb", [4, 128, 128], fp8, kind="ExternalInput").ap()
    esx = nc.dram_tensor("esx", [2, 128, 1], f32, kind="ExternalInput").ap()
    stx = nc.dram_tensor("stx", [2, 128, 1], f32, kind="ExternalInput").ap()
    sums_out = nc.dram_tensor("sums", [1, 2 * BC], f32, kind="ExternalOutput").ap()

    with tile.TileContext(nc) as tc:
        with (
            tc.tile_pool(name="consts", bufs=1) as consts,
            tc.tile_pool(name="ebig", bufs=1) as ebig,
            tc.tile_pool(name="afp", bufs=3) as afp,
            tc.tile_pool(name="bxp", bufs=3) as bxp,
            tc.tile_pool(name="fin", bufs=1) as fin,
            tc.tile_pool(name="psf", bufs=3, space="PSUM") as psfp,
            tc.tile_pool(name="psb", bufs=3, space="PSUM") as psbp,
            tc.tile_pool(name="pssm", bufs=1, space="PSUM") as pssm,
        ):
            # ---- constants / weights ----
            ones128_bf = consts.tile([128, 1], bf16)
            nc.vector.memset(ones128_bf, 1.0)

            wtf, wtb = [], []
            for i in range(4):
                w = consts.tile([128, 128], fp8, tag=f"wf{i}")
                nc.sync.dma_start(out=w, in_=wf[i])
                wtf.append(w)
                w = consts.tile([128, 128], fp8, tag=f"wb{i}")
                nc.sync.dma_start(out=w, in_=wb[i])
                wtb.append(w)

            es, st = [], []
            for j in range(2):
                t_ = consts.tile([128, 1], f32, tag=f"es{j}")
                nc.sync.dma_start(out=t_, in_=esx[j])
                es.append(t_)
                t_ = consts.tile([128, 1], f32, tag=f"st{j}")
                nc.sync.dma_start(out=t_, in_=stx[j])
                st.append(t_)

            # ---- E chunks: straight DMA into persistent bf16 tiles ----
            efch, ebch = [None] * nch, [None] * nch
            for c in range(nch):
                t0 = c * CH
                e_ = ebig.tile([128, CH, 2 * BC], bf16, tag=f"ef{c}")
                nc.sync.dma_start(out=e_, in_=ef[:, t0:t0 + CH, :])
                efch[c] = e_
                e_ = ebig.tile([128, CH, 2 * BC], bf16, tag=f"eb{c}")
                nc.sync.dma_start(out=e_, in_=eb[:, t0:t0 + CH, :])
                ebch[c] = e_

            def efs(r):
                return efch[r // CH][:, r % CH, :]

            def ebs(r):
                return ebch[r // CH][:, r % CH, :]

            # ---- init states ----
            # A_0 = E'_0 * exp(trans[:, START])  (per tag-half j)
            af = afp.tile([128, 2 * BC], bf16, tag="af")
            for j in range(2):
                nc.vector.tensor_scalar_mul(
                    af[:, j * BC:(j + 1) * BC], efs(0)[:, j * BC:(j + 1) * BC], es[j])
            # X_0 = beta_511 * E'_511 = stop * E'b[0]
            bx = bxp.tile([128, 2 * BC], bf16, tag="bx")
            for j in range(2):
                nc.vector.tensor_scalar_mul(
                    bx[:, j * BC:(j + 1) * BC], ebs(0)[:, j * BC:(j + 1) * BC], st[j])

            # ---- scan: 256 rounds, two chains ----
            # Emission order is bwd-first on BOTH engines so the engine
            # FIFOs keep each chain's round at its queue head: the bwd
            # critical cycle is TTb -> bwd MMs -> TTb with the fwd chain's
            # work slotting into the latency gaps (and vice versa).
            for r in range(1, R + 1):
                # bwd: beta = W^T @ X   (consumes bx = X^{(r-1)})
                psb = psbp.tile([128, 2 * BC], f32, tag="psb")
                for jo in range(2):
                    o = psb[:, jo * BC:(jo + 1) * BC]
                    nc.tensor.matmul(o, wtb[0 * 2 + jo], bx[:, 0:BC],
                                     start=True, stop=False)
                    nc.tensor.matmul(o, wtb[1 * 2 + jo], bx[:, BC:2 * BC],
                                     start=False, stop=True)
                if r <= R - 1:
                    bx_new = bxp.tile([128, 2 * BC], bf16, tag="bx")
                    nc.vector.tensor_mul(bx_new, psb, ebs(r))
                    bx = bx_new
                    # fwd: P = W @ A_{r-1}
                    psf = psfp.tile([128, 2 * BC], f32, tag="psf")
                    for jo in range(2):
                        o = psf[:, jo * BC:(jo + 1) * BC]
                        nc.tensor.matmul(o, wtf[0 * 2 + jo], af[:, 0:BC],
                                         start=True, stop=False)
                        nc.tensor.matmul(o, wtf[1 * 2 + jo], af[:, BC:2 * BC],
                                         start=False, stop=True)
                    af_new = afp.tile([128, 2 * BC], bf16, tag="af")
                    nc.vector.tensor_mul(af_new, psf, efs(r))
                    af = af_new
                else:
                    # join: S_jb = sum_p A_255 * beta_255   (beta_255 = psb)
                    tj = fin.tile([128, 2 * BC], bf16, tag="tj")
                    nc.vector.tensor_mul(tj, psb, af)
                    sums_ps = pssm.tile([1, 2 * BC], f32, tag="sums")
                    nc.tensor.matmul(sums_ps, ones128_bf, tj,
                                     start=True, stop=True)
                    sums_sb = fin.tile([1, 2 * BC], f32, tag="sums_sb")
                    nc.vector.tensor_copy(sums_sb, sums_ps)
                    nc.sync.dma_start(out=sums_out, in_=sums_sb)

    nc.compile()
    return nc


_PROG_CACHE = {}


def _get_program():
    if "p" not in _PROG_CACHE:
        _PROG_CACHE["p"] = _build_program()
    return _PROG_CACHE["p"]


def _host_numerator(inputs, transitions, tags, mask):
    fm = mask.astype(np.float32)
    score = transitions[tags[:, 0], START].astype(np.float32)
    trans_sc = transitions[tags[:, 1:], tags[:, :-1]] * fm[:, 1:]
    emit_sc = np.take_along_axis(
        inputs[:, :-1, :], tags[:, :-1, None], axis=2)[..., 0] * fm[:, :-1]
    score = score + trans_sc.sum(-1) + emit_sc.sum(-1)
    last_idx = (fm.sum(-1) - 1.0).astype(np.int32)
    last_tags = np.take_along_axis(tags, last_idx[:, None], axis=1)[:, 0]
    last_input = np.take_along_axis(
        inputs[:, -1, :], last_tags[:, None], axis=1)[:, 0]
    return score + transitions[STOP, last_tags] + last_input * fm[:, -1]


def _make_in_maps(inputs, transitions):
    bf = ml_dtypes.bfloat16
    fp8 = ml_dtypes.float8_e4m3

    # E' = exp(x - C*ln2) as bf16, laid out [tag_part, round, (j, b)]
    ex = np.exp(inputs.astype(np.float32) - np.float32(SCALE_BITS * LN2))
    # v[j, p, t, b] with tag = j*128 + p
    v = ex.transpose(2, 1, 0).reshape(2, 128, T, B)
    tc = np.maximum(transitions, -100.0).astype(np.float32)
    expt = np.exp(tc)  # W[next, prev]
    # fwd lhsT (k*2+j): W^T slice  [prev k-half, next j-half]
    wfs = np.ascontiguousarray(
        expt.T.reshape(2, 128, 2, 128).transpose(0, 2, 1, 3)
    ).reshape(4, 128, 128).astype(fp8)
    # bwd lhsT (k*2+j): W slice    [next k-half, prev j-half]
    wbs = np.ascontiguousarray(
        expt.reshape(2, 128, 2, 128).transpose(0, 2, 1, 3)
    ).reshape(4, 128, 128).astype(fp8)
    esx = np.exp(np.maximum(transitions[:, START], -100.0)
                 ).astype(np.float32).reshape(2, 128, 1)
    stx = np.exp(np.maximum(transitions[STOP, :], -100.0)
                 ).astype(np.float32).reshape(2, 128, 1)

    in_maps = []
    for c in range(NCORES):
        vc = v[:, :, :, c * BC:(c + 1) * BC]          # [2, 128, T, BC]
        efc = np.ascontiguousarray(
            vc[:, :, 0:R, :].transpose(1, 2, 0, 3)).reshape(128, R, 2 * BC)
        ebc = np.ascontiguousarray(
            vc[:, :, T - 1:R - 1:-1, :].transpose(1, 2, 0, 3)
        ).reshape(128, R, 2 * BC)
        in_maps.append({
            "ef": efc.astype(bf), "eb": ebc.astype(bf),
            "wf": wfs, "wb": wbs, "esx": esx, "stx": stx,
        })
    return in_maps


def kernel(inputs, transitions, tags, mask, _trace=False):
    from concourse.bass_utils import run_bass_kernel_spmd

    inputs = np.asarray(inputs, dtype=np.float32)
    transitions = np.asarray(transitions, dtype=np.float32)
    tags = np.asarray(tags)
    mask = np.asarray(mask)

    nc = _get_program()
    in_maps = _make_in_maps(inputs, transitions)
    res = run_bass_kernel_spmd(nc, in_maps, list(range(NCORES)), trace=_trace)
    sums = np.stack([r["sums"].reshape(2, BC) for r in res.results])  # (8,2,BC)
    S = (sums[:, 0, :] + sums[:, 1, :]).reshape(-1).astype(np.float64)
    denoms = np.log(S) + (2 * R) * SCALE_BITS * LN2

    num = _host_numerator(inputs, transitions, tags, mask).astype(np.float64)
    out = np.float32(np.sum(num - denoms))
    if _trace:
        return out, res
    return out


# revision 4
# speedup vs baseline: 3.5681x; 1.6618x over previous
"""CRF loss (nn_ConditionalRandomField) Bass/Trainium2 kernel — v3.

Strategy
--------
loss = sum_b (numerator[b] - log_denominator[b])

The denominator is a length-512 sequential scan A_t = (W @ A_{t-1}) * E_t
(exp space, W = exp(transitions), E_t = exp(inputs_t) * 2^-C with a
constant per-step prescale C=9.42 — no data-dependent renorm needed; the
exact correction 512*C*ln2 is added back on the host).

The per-round latency cycle on TRN2 (PSUM-access TT + sem + matmul drain
+ sem ~ 630ns) is fixed by hardware, so the win comes from cutting the
ROUND COUNT: time is split into K=8 segments of 64 steps. Products of 64
positive transfer matrices are numerically rank-1 (Birkhoff contraction),
so middle segments are summarized by one forward scan u_s = M_s @ 1 and
one backward scan w_s = M_s^T @ 1, joined by dot products:

  denom ~= ln[ (w2.y1) * prod_s (w_{s+1}.u_s) * (b8.u7) / prod_s (1.u_s) ]

All 7 forward chains share the same weights, so they PACK into the free
dim of the same matmuls (state [128, 7x64], FD=224 per MM, one
tensor_tensor [128,448] per direction per round) — 64 rounds total.

Per core (batch-parallel, 32 sequences each): per round 8 matmuls
(2 directions x 2 out-halves x 2 k-tiles) + 2 vector multiplies.
Weights are fp8e4 (exp'd on host), state bf16, PSUM f32.
numerator is a tiny O(B*T) gather — computed on host in numpy.
"""

import numpy as np
import ml_dtypes

B, T, N = 256, 512, 256
START, STOP = 254, 255
NCORES = 8
BC = B // NCORES          # 32 sequences per core
K = 8                     # time segments
L = T // K                # 64 steps per segment = rounds
NCH = 7                   # chains per direction (fwd: seg1..7, bwd: seg2..8)
FD = NCH * 2 * BC         # 448 packed state columns
DCH = 8                   # DMA chunk size in rounds
SCALE_BITS = 9.42
LN2 = float(np.log(2.0))


def _build_program():
    import concourse.bass as bass  # noqa: F401
    import concourse.tile as tile
    from concourse import bacc, mybir

    f32 = mybir.dt.float32
    bf16 = mybir.dt.bfloat16
    fp8 = mybir.dt.float8e4

    nc = bacc.Bacc("TRN2", target_bir_lowering=False, debug=False,
                   enable_asserts=False)

    ef = nc.dram_tensor("ef", [128, L, FD], bf16, kind="ExternalInput").ap()
    eb = nc.dram_tensor("eb", [128, L, FD], bf16, kind="ExternalInput").ap()
    efinit = nc.dram_tensor("efinit", [128, 2 * BC], bf16,
                            kind="ExternalInput").ap()
    ebinit = nc.dram_tensor("ebinit", [128, FD], bf16,
                            kind="ExternalInput").ap()
    wf = nc.dram_tensor("wf", [4, 128, 128], fp8, kind="ExternalInput").ap()
    wb = nc.dram_tensor("wb", [4, 128, 128], fp8, kind="ExternalInput").ap()
    esx = nc.dram_tensor("esx", [2, 128, 1], f32, kind="ExternalInput").ap()
    stx = nc.dram_tensor("stx", [2, 128, 1], f32, kind="ExternalInput").ap()
    sums_out = nc.dram_tensor("sums", [1, FD + (NCH - 1) * 2 * BC], f32,
                              kind="ExternalOutput").ap()

    with tile.TileContext(nc) as tc:
        with (
            tc.tile_pool(name="consts", bufs=1) as consts,
            tc.tile_pool(name="ebig", bufs=1) as ebig,
            tc.tile_pool(name="afp", bufs=3) as afp,
            tc.tile_pool(name="bxp", bufs=3) as bxp,
            tc.tile_pool(name="fin", bufs=1) as fin,
            tc.tile_pool(name="psf", bufs=2, space="PSUM") as psfp,
            tc.tile_pool(name="psb", bufs=2, space="PSUM") as psbp,
            tc.tile_pool(name="pss1", bufs=1, space="PSUM") as pss1,
            tc.tile_pool(name="pss2", bufs=1, space="PSUM") as pss2,
        ):
            ones128_bf = consts.tile([128, 1], bf16)
            nc.vector.memset(ones128_bf, 1.0)

            wtf, wtb = [], []
            for i in range(4):
                w = consts.tile([128, 128], fp8, tag=f"wf{i}")
                nc.sync.dma_start(out=w, in_=wf[i])
                wtf.append(w)
                w = consts.tile([128, 128], fp8, tag=f"wb{i}")
                nc.sync.dma_start(out=w, in_=wb[i])
                wtb.append(w)

            es, st = [], []
            for j in range(2):
                t_ = consts.tile([128, 1], f32, tag=f"es{j}")
                nc.sync.dma_start(out=t_, in_=esx[j])
                es.append(t_)
                t_ = consts.tile([128, 1], f32, tag=f"st{j}")
                nc.sync.dma_start(out=t_, in_=stx[j])
                st.append(t_)

            einit_t = consts.tile([128, 2 * BC], bf16, tag="efinit")
            nc.sync.dma_start(out=einit_t, in_=efinit)
            binit_t = consts.tile([128, FD], bf16, tag="ebinit")
            nc.sync.dma_start(out=binit_t, in_=ebinit)

            # E chunks land as [128, DCH*NCH, 2*BC] tiles (same bytes as
            # the [128, DCH, FD] dram slice); slice i of chunk c is
            # [:, q*NCH:(q+1)*NCH, :] with q = i - c*DCH.
            nchunks = L // DCH
            efch, ebch = [None] * nchunks, [None] * nchunks
            for c in range(nchunks):
                e_ = ebig.tile([128, DCH * NCH, 2 * BC], bf16, tag=f"ef{c}")
                nc.sync.dma_start(out=e_, in_=ef[:, c * DCH:(c + 1) * DCH, :])
                efch[c] = e_
                e_ = ebig.tile([128, DCH * NCH, 2 * BC], bf16, tag=f"eb{c}")
                nc.sync.dma_start(out=e_, in_=eb[:, c * DCH:(c + 1) * DCH, :])
                ebch[c] = e_

            def efs(i):
                q = i % DCH
                return efch[i // DCH][:, q * NCH:(q + 1) * NCH, :]

            def ebs(i):
                q = i % DCH
                return ebch[i // DCH][:, q * NCH:(q + 1) * NCH, :]

            # ---- init packed states ----
            # AF block 0 = A_0 = E'_0 * exp(trans[:, START]); blocks 1..6 = 1
            af = afp.tile([128, NCH, 2 * BC], bf16, tag="af")
            for j in range(2):
                nc.vector.tensor_scalar_mul(
                    af[:, 0, j * BC:(j + 1) * BC],
                    einit_t[:, j * BC:(j + 1) * BC], es[j])
            nc.vector.memset(af[:, 1:NCH, :], 1.0)
            # BX blocks 0..5 (segs 2..7) = E'_{64s-1} * 1; block 6 (seg8)
            # = E'_511 * exp(trans[STOP, :])
            bx = bxp.tile([128, NCH, 2 * BC], bf16, tag="bx")
            nc.vector.tensor_copy(bx[:, 0:NCH - 1, :],
                                  binit_t[:, 0:(NCH - 1) * 2 * BC])
            for j in range(2):
                nc.vector.tensor_scalar_mul(
                    bx[:, NCH - 1, j * BC:(j + 1) * BC],
                    binit_t[:, (NCH - 1) * 2 * BC + j * BC:
                            (NCH - 1) * 2 * BC + (j + 1) * BC], st[j])

            # ---- scan: 64 rounds ----
            af63 = None
            for r in range(1, L + 1):
                # bwd MMs: beta = W^T @ X
                psb = psbp.tile([128, NCH, 2 * BC], f32, tag="psb")
                for jo in range(2):
                    o = psb[:, :, jo * BC:(jo + 1) * BC]
                    nc.tensor.matmul(o, wtb[0 * 2 + jo], bx[:, :, 0:BC],
                                     start=True, stop=False)
                    nc.tensor.matmul(o, wtb[1 * 2 + jo], bx[:, :, BC:2 * BC],
                                     start=False, stop=True)
                if r <= L - 1:
                    bx_new = bxp.tile([128, NCH, 2 * BC], bf16, tag="bx")
                    nc.vector.tensor_mul(bx_new, psb, ebs(r - 1))
                    bx = bx_new
                    # fwd MMs: P = W @ A
                    psf = psfp.tile([128, NCH, 2 * BC], f32, tag="psf")
                    for jo in range(2):
                        o = psf[:, :, jo * BC:(jo + 1) * BC]
                        nc.tensor.matmul(o, wtf[0 * 2 + jo], af[:, :, 0:BC],
                                         start=True, stop=False)
                        nc.tensor.matmul(o, wtf[1 * 2 + jo],
                                         af[:, :, BC:2 * BC],
                                         start=False, stop=True)
                    af_new = afp.tile([128, NCH, 2 * BC], bf16, tag="af")
                    nc.vector.tensor_mul(af_new, psf, efs(r - 1))
                    af = af_new
                else:
                    # r == L: bwd MM only (psb = final betas); fwd advances
                    # only the middle chains (seg1 stopped at y1 = A_63).
                    af63 = af
                    psf = psfp.tile([128, NCH - 1, 2 * BC], f32, tag="psf")
                    for jo in range(2):
                        o = psf[:, :, jo * BC:(jo + 1) * BC]
                        nc.tensor.matmul(o, wtf[0 * 2 + jo],
                                         af[:, 1:NCH, 0:BC],
                                         start=True, stop=False)
                        nc.tensor.matmul(o, wtf[1 * 2 + jo],
                                         af[:, 1:NCH, BC:2 * BC],
                                         start=False, stop=True)
                    af_mid = fin.tile([128, NCH - 1, 2 * BC], bf16,
                                      tag="af_mid")
                    nc.vector.tensor_mul(af_mid, psf,
                                         efs(L - 1)[:, 1:NCH, :])

            # ---- join ----
            # tj block i = (bwd seg_{i+2} final beta) * (fwd final state)
            tj = fin.tile([128, NCH, 2 * BC], bf16, tag="tj")
            nc.vector.tensor_mul(tj[:, 0, :], psb[:, 0, :], af63[:, 0, :])
            nc.vector.tensor_mul(tj[:, 1:NCH, :], psb[:, 1:NCH, :], af_mid)
            s1 = pss1.tile([1, FD], f32, tag="s1")
            nc.tensor.matmul(s1, ones128_bf, tj, start=True, stop=True)
            s2 = pss2.tile([1, (NCH - 1) * 2 * BC], f32, tag="s2")
            nc.tensor.matmul(s2, ones128_bf, af_mid, start=True, stop=True)
            sums_sb = fin.tile([1, FD + (NCH - 1) * 2 * BC], f32, tag="sums")
            nc.vector.tensor_copy(sums_sb[:, 0:FD], s1)
            nc.vector.tensor_copy(sums_sb[:, FD:], s2)
            nc.sync.dma_start(out=sums_out, in_=sums_sb)

    nc.compile()
    return nc


_PROG_CACHE = {}


def _get_program():
    if "p" not in _PROG_CACHE:
        _PROG_CACHE["p"] = _build_program()
    return _PROG_CACHE["p"]


def _host_numerator(inputs, transitions, tags, mask):
    fm = mask.astype(np.float32)
    score = transitions[tags[:, 0], START].astype(np.float32)
    trans_sc = transitions[tags[:, 1:], tags[:, :-1]] * fm[:, 1:]
    emit_sc = np.take_along_axis(
        inputs[:, :-1, :], tags[:, :-1, None], axis=2)[..., 0] * fm[:, :-1]
    score = score + trans_sc.sum(-1) + emit_sc.sum(-1)
    last_idx = (fm.sum(-1) - 1.0).astype(np.int32)
    last_tags = np.take_along_axis(tags, last_idx[:, None], axis=1)[:, 0]
    last_input = np.take_along_axis(
        inputs[:, -1, :], last_tags[:, None], axis=1)[:, 0]
    return score + transitions[STOP, last_tags] + last_input * fm[:, -1]


def _make_in_maps(inputs, transitions):
    bf = ml_dtypes.bfloat16
    fp8 = ml_dtypes.float8_e4m3

    ex = np.exp(inputs.astype(np.float32) - np.float32(SCALE_BITS * LN2))
    v = ex.transpose(2, 1, 0).reshape(2, 128, T, B)   # [j, p, t, b]

    # fwd slice i, chain block ci: seg1 (ci=0) -> E'_{i+1} (i<=62);
    # middles (ci>=1, seg s=ci+1) -> E'_{64*ci+i}
    tf = np.zeros((L, NCH), np.int64)
    tf[:L - 1, 0] = np.arange(1, L)
    for ci in range(1, NCH):
        tf[:, ci] = 64 * ci + np.arange(L)
    # bwd slice i, block ci (seg s=ci+2): E'_{64*ci+126-i} (i<=62)
    tb = np.zeros((L, NCH), np.int64)
    for ci in range(NCH):
        tb[:L - 1, ci] = 64 * ci + 126 - np.arange(L - 1)

    tc_ = np.maximum(transitions, -100.0).astype(np.float32)
    expt = np.exp(tc_)
    wfs = np.ascontiguousarray(
        expt.T.reshape(2, 128, 2, 128).transpose(0, 2, 1, 3)
    ).reshape(4, 128, 128).astype(fp8)
    wbs = np.ascontiguousarray(
        expt.reshape(2, 128, 2, 128).transpose(0, 2, 1, 3)
    ).reshape(4, 128, 128).astype(fp8)
    esx = np.exp(np.maximum(transitions[:, START], -100.0)
                 ).astype(np.float32).reshape(2, 128, 1)
    stx = np.exp(np.maximum(transitions[STOP, :], -100.0)
                 ).astype(np.float32).reshape(2, 128, 1)

    in_maps = []
    for c in range(NCORES):
        vc = v[:, :, :, c * BC:(c + 1) * BC]          # [2, 128, T, BC]
        efc = np.ascontiguousarray(
            vc[:, :, tf, :].transpose(1, 2, 3, 0, 4)).reshape(128, L, FD)
        efc[:, L - 1, 0:2 * BC] = 0.0
        ebc = np.ascontiguousarray(
            vc[:, :, tb, :].transpose(1, 2, 3, 0, 4)).reshape(128, L, FD)
        ebc[:, L - 1, :] = 0.0
        efinit = np.ascontiguousarray(
            vc[:, :, 0, :].transpose(1, 0, 2)).reshape(128, 2 * BC)
        t_init = 64 * (np.arange(NCH) + 2) - 1        # E'_{64s-1}, s=2..8
        ebinit = np.ascontiguousarray(
            vc[:, :, t_init, :].transpose(1, 2, 0, 3)).reshape(128, FD)
        in_maps.append({
            "ef": efc.astype(bf), "eb": ebc.astype(bf),
            "efinit": efinit.astype(bf), "ebinit": ebinit.astype(bf),
            "wf": wfs, "wb": wbs, "esx": esx, "stx": stx,
        })
    return in_maps


def kernel(inputs, transitions, tags, mask, _trace=False):
    from concourse.bass_utils import run_bass_kernel_spmd

    inputs = np.asarray(inputs, dtype=np.float32)
    transitions = np.asarray(transitions, dtype=np.float32)
    tags = np.asarray(tags)
    mask = np.asarray(mask)

    nc = _get_program()
    in_maps = _make_in_maps(inputs, transitions)
    res = run_bass_kernel_spmd(nc, in_maps, list(range(NCORES)), trace=_trace)

    denoms = np.empty(B, np.float64)
    for c in range(NCORES):
        s = res.results[c]["sums"].reshape(-1).astype(np.float64)
        s1 = s[0:FD].reshape(NCH, 2, BC)              # (w_{s}.u) dots
        s2 = s[FD:].reshape(NCH - 1, 2, BC)           # (1.u_s) dots
        S1 = s1[:, 0, :] + s1[:, 1, :]                # (7, BC)
        S2 = s2[:, 0, :] + s2[:, 1, :]                # (6, BC)
        denoms[c * BC:(c + 1) * BC] = (
            np.log(S1).sum(0) - np.log(S2).sum(0)
            + T * SCALE_BITS * LN2)

    num = _host_numerator(inputs, transitions, tags, mask).astype(np.float64)
    out = np.float32(np.sum(num - denoms))
    if _trace:
        return out, res
    return out
